# revision 9
# baseline (speedup 1.0000x reference)
"""Trainium2 Bass kernel for DMS_STAttention (gnn_message_passing).

Self-contained: builds host-side constants, compiles one SPMD NEFF, shards
batch B=256 across 8 NeuronCores (32 batches each), runs via
run_bass_kernel_spmd, returns (sa, ta) like the reference.
"""
import numpy as np

# ---- problem shapes (hardcoded) ----
B, C, T, J = 256, 64, 64, 25
NCORES = 8
BPC = B // NCORES            # 32 batches per core
NPAIR = BPC // 2             # 16 slab pairs per core
S_SCALES = [25, 12, 5]
T_SCALES = [64, 32, 16]
LEAKY = 0.2

F32 = None  # set after mybir import


def _host_consts(inputs):
    """Precompute small weights on host (fp32 numpy)."""
    W = np.asarray(inputs['W'], np.float32)
    c = {}
    for br, pre in (('s', 'as'), ('t', 'at')):
        for i in range(3):
            ws = (W @ np.asarray(inputs[f'{pre}_src{i}'], np.float32)).ravel()
            wt = (W @ np.asarray(inputs[f'{pre}_dst{i}'], np.float32)).ravel()
            # scaled by LEAKY: Mtilde = 0.2*(s+t); attn = Mtilde + 4*relu(Mtilde)
            wst = np.stack([ws, wt], 1) * LEAKY                     # [64,2]
            c[f'wcol_{br}{i}'] = np.ascontiguousarray(np.concatenate([wst, wst], 0), np.float32)
            # replicated: cols 0-63 = ws_scaled, 64-127 = wt_scaled
            rep = np.concatenate([np.tile(wst[:, 0:1], (1, 64)),
                                  np.tile(wst[:, 1:2], (1, 64))], 1)
            c[f'wrep_{br}{i}'] = np.ascontiguousarray(np.concatenate([rep, rep], 0), np.float32)
    c['wp_s0'] = np.asarray(inputs['Wsp0'], np.float32)             # [64,12]
    c['wp_s1'] = np.asarray(inputs['Wsp1'], np.float32)             # [64,5]
    c['wp_t0'] = np.asarray(inputs['Wtp0'], np.float32)             # [64,32]
    c['wp_t1'] = np.asarray(inputs['Wtp1'], np.float32)             # [64,16]
    # bias rows replicated across partitions [64, n1]
    c['b_s0'] = np.tile(np.asarray(inputs['bsp0'], np.float32)[None, :], (64, 1))
    c['b_s1'] = np.tile(np.asarray(inputs['bsp1'], np.float32)[None, :], (64, 1))
    c['b_t0'] = np.tile(np.asarray(inputs['btp0'], np.float32)[None, :], (64, 1))
    c['b_t1'] = np.tile(np.asarray(inputs['btp1'], np.float32)[None, :], (64, 1))
    c['ident'] = np.concatenate([np.eye(64, dtype=np.float32)] * 2, 0)
    return c


CONST_SHAPES = {
    **{f'wcol_{b}{i}': (128, 2) for b in 'st' for i in range(3)},
    **{f'wrep_{b}{i}': (128, 128) for b in 'st' for i in range(3)},
    'wp_s0': (64, 12), 'wp_s1': (64, 5), 'wp_t0': (64, 32), 'wp_t1': (64, 16),
    'b_s0': (64, 12), 'b_s1': (64, 5), 'b_t0': (64, 32), 'b_t1': (64, 16),
    'ident': (128, 64),
}


def _lrelu_attn(nc, pools, rows_psum, cols_sb, col_idx_stride, out_sb, P_rows, G, W):
    """attn^T tile [P_rows, G*W] = lrelu(s+t) given Mtilde parts.

    rows_psum: [P_rows, G*W] psum AP holding 0.2*s rows (free-varying)
    cols_sb:   SBUF AP [P_rows, G] holding 0.2*t cols per graph
    out_sb:    [P_rows, G*W] SBUF tile for attn result
    """
    import concourse.mybir as mybir
    sb = pools['work']
    m = sb.tile([P_rows, G * W], F32, tag=f'mt{P_rows}_{W}')
    nc.vector.tensor_tensor(
        out=m[:].rearrange("p (g w) -> p g w", g=G),
        in0=rows_psum.rearrange("p (g w) -> p g w", g=G),
        in1=cols_sb.unsqueeze(2).broadcast_to([P_rows, G, W]),
        op=mybir.AluOpType.add)
    r = sb.tile([P_rows, G * W], F32, tag=f'r4{P_rows}_{W}')
    nc.scalar.activation(r[:], m[:], mybir.ActivationFunctionType.Relu, scale=4.0)
    nc.gpsimd.tensor_tensor(out=out_sb[:], in0=m[:], in1=r[:], op=mybir.AluOpType.add)


def _softmax(nc, pools, z_psum, brep, n1, G, s_out, tagp):
    """Row softmax over z (+bias brep) : z_psum [P, G*n1] -> s_out SBUF."""
    import concourse.mybir as mybir
    sb = pools['work']
    P = z_psum.shape[0]
    if brep is not None:
        zb = sb.tile([P, G * n1], F32, tag=f'zb{tagp}')
        nc.vector.tensor_tensor(
            out=zb[:].rearrange("p (g w) -> p g w", g=G),
            in0=z_psum.rearrange("p (g w) -> p g w", g=G),
            in1=brep[0:P, :].unsqueeze(1).broadcast_to([P, G, n1]),
            op=mybir.AluOpType.add)
        zsrc = zb[:]
    else:
        zsrc = z_psum
    nm = sb.tile([P, G], F32, tag=f'nm{tagp}')
    nc.vector.tensor_reduce(nm[:], zsrc.rearrange("p (g w) -> p g w", g=G),
                            axis=mybir.AxisListType.X, op=mybir.AluOpType.max,
                            negate=True)
    zs = sb.tile([P, G * n1], F32, tag=f'zs{tagp}')
    nc.vector.tensor_tensor(
        out=zs[:].rearrange("p (g w) -> p g w", g=G),
        in0=zsrc.rearrange("p (g w) -> p g w", g=G),
        in1=nm[:].unsqueeze(2).broadcast_to([P, G, n1]),
        op=mybir.AluOpType.add)
    nc.scalar.activation(zs[:], zs[:], mybir.ActivationFunctionType.Exp)
    r = sb.tile([P, G], F32, tag=f'sr{tagp}')
    nc.vector.tensor_reduce(r[:], zs[:].rearrange("p (g w) -> p g w", g=G),
                            axis=mybir.AxisListType.X, op=mybir.AluOpType.add)
    nc.vector.reciprocal(r[:], r[:])
    nc.vector.tensor_tensor(
        out=s_out[:].rearrange("p (g w) -> p g w", g=G),
        in0=zs[:].rearrange("p (g w) -> p g w", g=G),
        in1=r[:].unsqueeze(2).broadcast_to([P, G, n1]),
        op=mybir.AluOpType.mult)


def _branch_block(nc, pools, cst, br, xnm_views, xcm_views, out_stage, out_off,
                  G, scales, ident, cbase=0):
    """Process one block of G graphs for branch br ('s'|'t').

    xnm_views[g]: SBUF AP [n, 64] node-major x
    xcm_views[g]: SBUF AP [64, n] channel-major x (for stCols/sRep)
    out_stage: SBUF staging AP [n, ...]; out_off: column offset (elements)
    """
    import concourse.mybir as mybir
    n0, n1, n2 = scales
    sb, ps = pools['work'], pools['psum']

    # ---- scale 0 reps + cols ----
    strep = ps.tile([128, G * n0], F32, tag='rep')
    # one matmul per block: rhs = concat of xcm views is not one AP; instead
    # issue per-graph matmuls into strep slices (lhsT shared const).
    for g in range(G):
        nc.tensor.matmul(strep[:, g * n0:(g + 1) * n0],
                         cst[f'wrep_{br}0'][cbase:cbase + 64, :],
                         xcm_views[g], start=True, stop=True)
    stc = ps.tile([n0, 2 * G], F32, tag='pstc')
    for g in range(G):
        nc.tensor.matmul(stc[:, 2 * g:2 * g + 2], xcm_views[g],
                         cst[f'wcol_{br}0'][cbase:cbase + 64, :], start=True, stop=True)
    stcs = sb.tile([n0, 2 * G], F32, tag='stcs')
    nc.scalar.copy(stcs[:], stc[:])
    scols = stcs[:].rearrange("p (g two) -> p g two", two=2)[:, :, 0]
    tcols = stcs[:].rearrange("p (g two) -> p g two", two=2)[:, :, 1]

    attn0T = sb.tile([n0, G * n0], F32, tag='attn0T')
    _lrelu_attn(nc, pools, strep[0:n0, :], tcols, 2, attn0T, n0, G, n0)

    # ---- A0x^T, Z0, S0 ----
    a0 = ps.tile([64, G * n0], F32, tag='pA')
    for g in range(G):
        nc.tensor.matmul(a0[:, g * n0:(g + 1) * n0], xnm_views[g],
                         attn0T[:, g * n0:(g + 1) * n0], start=True, stop=True)
    a0s = sb.tile([64, G * n0], F32, tag='a0s')
    nc.scalar.copy(a0s[:], a0[:])
    z0 = ps.tile([n0, G * n1], F32, tag='pZ')
    for g in range(G):
        nc.tensor.matmul(z0[:, g * n1:(g + 1) * n1], a0s[:, g * n0:(g + 1) * n0],
                         cst[f'wp_{br}0'][:], start=True, stop=True)
    s0 = sb.tile([n0, G * n1], F32, tag='s0')
    _softmax(nc, pools, z0[:], cst[f'b_{br}0'], n1, G, s0, 'z0')

    # S0^T
    s0t_p = ps.tile([n1, G * n0], F32, tag='pB')
    for g in range(G):
        nc.tensor.transpose(s0t_p[:, g * n0:(g + 1) * n0],
                            s0[:, g * n1:(g + 1) * n1], ident[0:n0, 0:n0])
    s0t = sb.tile([n1, G * n0], F32, tag='s0ts')
    nc.scalar.copy(s0t[:], s0t_p[:])

    # x1 = S0^T @ X  (node-major [n1, 64])
    x1_p = ps.tile([n1, G * 64], F32, tag='pC')
    for g in range(G):
        nc.tensor.matmul(x1_p[:, g * 64:(g + 1) * 64], s0[:, g * n1:(g + 1) * n1],
                         xnm_views[g], start=True, stop=True)
    x1 = sb.tile([n1, G * 64], F32, tag='x1s')
    nc.scalar.copy(x1[:], x1_p[:])
    # x1 channel-major
    x1c_p = ps.tile([64, G * n1], F32, tag='pB')
    for g in range(G):
        nc.tensor.transpose(x1c_p[:, g * n1:(g + 1) * n1],
                            x1[:, g * 64:(g + 1) * 64], ident[0:n1, 0:n1])
    x1c = sb.tile([64, G * n1], F32, tag='x1cs')
    nc.scalar.copy(x1c[:], x1c_p[:])

    # ---- scale 1 ----
    strep1 = ps.tile([128, G * n1], F32, tag='rep')
    for g in range(G):
        nc.tensor.matmul(strep1[:, g * n1:(g + 1) * n1],
                         cst[f'wrep_{br}1'][0:64, :],
                         x1c[:, g * n1:(g + 1) * n1], start=True, stop=True)
    stc1 = ps.tile([n1, 2 * G], F32, tag='pstc')
    for g in range(G):
        nc.tensor.matmul(stc1[:, 2 * g:2 * g + 2], x1c[:, g * n1:(g + 1) * n1],
                         cst[f'wcol_{br}1'][0:64, :], start=True, stop=True)
    stc1s = sb.tile([n1, 2 * G], F32, tag='stc1s')
    nc.scalar.copy(stc1s[:], stc1[:])
    tcols1 = stc1s[:].rearrange("p (g two) -> p g two", two=2)[:, :, 1]
    attn1T = sb.tile([n1, G * n1], F32, tag='attn1T')
    _lrelu_attn(nc, pools, strep1[0:n1, :], tcols1, 2, attn1T, n1, G, n1)

    a1 = ps.tile([64, G * n1], F32, tag='pA')
    for g in range(G):
        nc.tensor.matmul(a1[:, g * n1:(g + 1) * n1], x1[:, g * 64:(g + 1) * 64],
                         attn1T[:, g * n1:(g + 1) * n1], start=True, stop=True)
    a1s = sb.tile([64, G * n1], F32, tag='a1s')
    nc.scalar.copy(a1s[:], a1[:])
    z1 = ps.tile([n1, G * n2], F32, tag='pZ')
    for g in range(G):
        nc.tensor.matmul(z1[:, g * n2:(g + 1) * n2], a1s[:, g * n1:(g + 1) * n1],
                         cst[f'wp_{br}1'][:], start=True, stop=True)
    s1 = sb.tile([n1, G * n2], F32, tag='s1')
    _softmax(nc, pools, z1[:], cst[f'b_{br}1'], n2, G, s1, 'z1')

    s1t_p = ps.tile([n2, G * n1], F32, tag='s1t')
    for g in range(G):
        nc.tensor.transpose(s1t_p[:, g * n1:(g + 1) * n1],
                            s1[:, g * n2:(g + 1) * n2], ident[0:n1, 0:n1])
    s1t = sb.tile([n2, G * n1], F32, tag='s1ts')
    nc.scalar.copy(s1t[:], s1t_p[:])

    # x2 = S1^T @ x1 (node-major [n2, 64])
    x2_p = ps.tile([n2, G * 64], F32, tag='pC')
    for g in range(G):
        nc.tensor.matmul(x2_p[:, g * 64:(g + 1) * 64], s1[:, g * n2:(g + 1) * n2],
                         x1[:, g * 64:(g + 1) * 64], start=True, stop=True)
    x2 = sb.tile([n2, G * 64], F32, tag='x2s')
    nc.scalar.copy(x2[:], x2_p[:])
    x2c_p = ps.tile([64, G * n2], F32, tag='pB')
    for g in range(G):
        nc.tensor.transpose(x2c_p[:, g * n2:(g + 1) * n2],
                            x2[:, g * 64:(g + 1) * 64], ident[0:n2, 0:n2])
    x2c = sb.tile([64, G * n2], F32, tag='x2cs')
    nc.scalar.copy(x2c[:], x2c_p[:])

    # ---- scale 2 attn ----
    strep2 = ps.tile([128, G * n2], F32, tag='rep')
    for g in range(G):
        nc.tensor.matmul(strep2[:, g * n2:(g + 1) * n2],
                         cst[f'wrep_{br}2'][0:64, :],
                         x2c[:, g * n2:(g + 1) * n2], start=True, stop=True)
    stc2 = ps.tile([n2, 2 * G], F32, tag='pstc')
    for g in range(G):
        nc.tensor.matmul(stc2[:, 2 * g:2 * g + 2], x2c[:, g * n2:(g + 1) * n2],
                         cst[f'wcol_{br}2'][0:64, :], start=True, stop=True)
    stc2s = sb.tile([n2, 2 * G], F32, tag='stc2s')
    nc.scalar.copy(stc2s[:], stc2[:])
    tcols2 = stc2s[:].rearrange("p (g two) -> p g two", two=2)[:, :, 1]
    attn2T = sb.tile([n2, G * n2], F32, tag='attn2T')
    _lrelu_attn(nc, pools, strep2[0:n2, :], tcols2, 2, attn2T, n2, G, n2)

    # ---- fusion: inner^T = attn1T + (S1 attn2T) S1T ; F = attn0 + S0 inner S0T
    dt_p = ps.tile([n2, G * n1], F32, tag='pA')
    for g in range(G):
        nc.tensor.matmul(dt_p[:, g * n1:(g + 1) * n1],
                         attn2T[:, g * n2:(g + 1) * n2],
                         s1t[:, g * n1:(g + 1) * n1], start=True, stop=True)
    dts = sb.tile([n2, G * n1], F32, tag='dts')
    nc.scalar.copy(dts[:], dt_p[:])
    it_p = ps.tile([n1, G * n1], F32, tag='pB')
    for g in range(G):
        nc.tensor.matmul(it_p[:, g * n1:(g + 1) * n1],
                         dts[:, g * n1:(g + 1) * n1],
                         s1t[:, g * n1:(g + 1) * n1], start=True, stop=True)
    its = sb.tile([n1, G * n1], F32, tag='its')
    nc.vector.tensor_tensor(out=its[:], in0=attn1T[:], in1=it_p[:],
                            op=mybir.AluOpType.add)
    ap_p = ps.tile([n1, G * n0], F32, tag='pC')
    for g in range(G):
        nc.tensor.matmul(ap_p[:, g * n0:(g + 1) * n0],
                         its[:, g * n1:(g + 1) * n1],
                         s0t[:, g * n0:(g + 1) * n0], start=True, stop=True)
    aps = sb.tile([n1, G * n0], F32, tag='apfs')
    nc.scalar.copy(aps[:], ap_p[:])
    f_p = ps.tile([n0, G * n0], F32, tag='pF')
    for g in range(G):
        nc.tensor.matmul(f_p[:, g * n0:(g + 1) * n0],
                         s0t[:, g * n0:(g + 1) * n0],
                         aps[:, g * n0:(g + 1) * n0], start=True, stop=False)
        nc.tensor.matmul(f_p[:, g * n0:(g + 1) * n0],
                         attn0T[:, g * n0:(g + 1) * n0],
                         ident[0:n0, 0:n0], is_transpose=True,
                         start=False, stop=True, skip_group_check=False)
    _softmax(nc, pools, f_p[:], None, n0, G,
             out_stage[:, out_off:out_off + G * n0], 'f' + br)


def build_program(nc, tc, ctx, n_pairs, use_loop):
    """Emit the full per-core program into nc/tc."""
    import concourse.mybir as mybir
    import concourse.bass as bass
    import concourse.tile as tile
    global F32
    F32 = mybir.dt.float32

    src = nc.dram_tensor("src", (BPC, C, T, J), F32, kind="ExternalInput").ap()
    sa = nc.dram_tensor("sa", (BPC, T, J, J), F32, kind="ExternalOutput").ap()
    ta = nc.dram_tensor("ta", (BPC, J, T, T), F32, kind="ExternalOutput").ap()
    cst_dram = {k: nc.dram_tensor(k, v, F32, kind="ExternalInput").ap()
                for k, v in CONST_SHAPES.items()}

    const_pool = ctx.enter_context(tc.tile_pool(name="const", bufs=1))
    io_pool = ctx.enter_context(tc.tile_pool(name="io", bufs=2))
    work = ctx.enter_context(tc.tile_pool(name="work", bufs=2))
    psum = ctx.enter_context(tc.tile_pool(name="psum", bufs=1, space="PSUM"))
    pools = {'work': work, 'psum': psum}

    cst = {}
    for k, shp in CONST_SHAPES.items():
        t = const_pool.tile(list(shp), F32, tag=k)
        nc.sync.dma_start(t[:], cst_dram[k][:])
        cst[k] = t
    ident = cst['ident']

    def body(pair_i):
        # ---- loads ----
        slab = io_pool.tile([128, T * J], F32, tag='slab')
        nc.sync.dma_start(slab[:], src[bass.ds(pair_i, 2)]
                          .rearrange("b c t j -> (b c) (t j)"))
        tpnm = io_pool.tile([T, 2 * C * J], F32, tag='tpnm')
        nc.sync.dma_start(tpnm[:].rearrange("t (b c j) -> t b c j", b=2, c=C),
                          src[bass.ds(pair_i, 2)].transpose([2, 0, 1, 3]))
        sa_stage = io_pool.tile([J, 2 * T * J], F32, tag='sastage')
        ta_stage = io_pool.tile([T, 2 * J * T], F32, tag='tastage')

        slab_v = slab[:].rearrange("p (t j) -> p t j", j=J)
        tpnm_v = tpnm[:].rearrange("p (b c j) -> p b c j", b=2, j=J)

        # ---- temporal branch: graphs (bl, j), blocks of G=5 over j ----
        Gt = 5
        for bl in range(2):
            for j0 in range(0, J, Gt):
                xnm = [tpnm_v[:, bl, :, j0 + g] for g in range(Gt)]
                # xnm view: [T, C] with free stride J
                xcm = [slab_v[bl * 64:(bl + 1) * 64, :, j0 + g] for g in range(Gt)]
                off = bl * (J * T) + j0 * T
                _branch_block(nc, pools, cst, 't', xnm, xcm, ta_stage, off,
                              Gt, T_SCALES, ident, cbase=bl * 64)
        # ---- spatial branch: graphs (bl, t), blocks of G=8 over t ----
        Gs = 8
        for bl in range(2):
            for t0 in range(0, T, Gs):
                xcm = [slab_v[bl * 64:(bl + 1) * 64, t0 + g, :] for g in range(Gs)]
                # x node-major via PE transpose of xcm
                xr_p = psum.tile([J, Gs * 64], F32, tag='rep')
                for g in range(Gs):
                    nc.tensor.transpose(xr_p[:, g * 64:(g + 1) * 64], xcm[g],
                                        ident[bl * 64:(bl + 1) * 64, :])
                xr = work.tile([J, Gs * 64], F32, tag='spxnms')
                nc.scalar.copy(xr[:], xr_p[:])
                xnm = [xr[:, g * 64:(g + 1) * 64] for g in range(Gs)]
                off = bl * (T * J) + t0 * J
                _branch_block(nc, pools, cst, 's', xnm, xcm, sa_stage, off,
                              Gs, S_SCALES, ident, cbase=bl * 64)

        # ---- stores ----
        nc.sync.dma_start(sa[bass.ds(pair_i, 2)].transpose([2, 0, 1, 3]),
                          sa_stage[:].rearrange("i (b t j) -> i b t j", b=2, t=T))
        nc.sync.dma_start(ta[bass.ds(pair_i, 2)].transpose([2, 0, 1, 3]),
                          ta_stage[:].rearrange("i (b j k) -> i b j k", b=2, j=J))

    if use_loop:
        with tc.For_i(0, 2 * n_pairs, 2) as i:
            body(i)
    else:
        for p in range(n_pairs):
            body(2 * p)


import concourse.mybir as mybir  # noqa: E402


def _build_nc(n_pairs=NPAIR, use_loop=True):
    from contextlib import ExitStack
    import concourse.bass as bass
    import concourse.tile as tile
    from concourse import bacc
    from concourse._compat import axon_active
    nc = bacc.Bacc("TRN2", target_bir_lowering=False, debug=False,
                   enable_asserts=False, num_devices=1)
    with tile.TileContext(nc) as tc:
        with ExitStack() as ctx:
            build_program(nc, tc, ctx, n_pairs, use_loop)
    nc.compile()
    return nc


_NC_CACHE = {}


def kernel(**inputs):
    from concourse.bass_utils import run_bass_kernel_spmd
    key = 'full'
    if key not in _NC_CACHE:
        _NC_CACHE[key] = _build_nc()
    nc = _NC_CACHE[key]
    cst = _host_consts(inputs)
    src = np.ascontiguousarray(np.asarray(inputs['src'], np.float32))
    in_maps = []
    for c in range(NCORES):
        m = {'src': np.ascontiguousarray(src[c * BPC:(c + 1) * BPC])}
        m.update(cst)
        in_maps.append(m)
    res = run_bass_kernel_spmd(nc, in_maps, core_ids=list(range(NCORES)))
    sa = np.concatenate([r['sa'] for r in res.results], 0)
    ta = np.concatenate([r['ta'] for r in res.results], 0)
    return sa, ta


# revision 11
# speedup vs baseline: 131.1253x; 131.1253x over previous
"""Trainium2 Bass kernel for DMS_STAttention (gnn_message_passing).

Self-contained: builds host-side constants, compiles one SPMD NEFF, shards
batch B=256 across 8 NeuronCores (32 batches each), runs via
run_bass_kernel_spmd, returns (sa, ta) like the reference.
"""
import numpy as np

# ---- problem shapes (hardcoded) ----
B, C, T, J = 256, 64, 64, 25
NCORES = 8
BPC = B // NCORES            # 32 batches per core
NPAIR = BPC // 2             # 16 slab pairs per core
S_SCALES = [25, 12, 5]
T_SCALES = [64, 32, 16]
LEAKY = 0.2

F32 = None  # set after mybir import


def _host_consts(inputs):
    """Precompute small weights on host (fp32 numpy)."""
    W = np.asarray(inputs['W'], np.float32)
    c = {}
    for br, pre in (('s', 'as'), ('t', 'at')):
        for i in range(3):
            ws = (W @ np.asarray(inputs[f'{pre}_src{i}'], np.float32)).ravel()
            wt = (W @ np.asarray(inputs[f'{pre}_dst{i}'], np.float32)).ravel()
            # scaled by LEAKY: Mtilde = 0.2*(s+t); attn = Mtilde + 4*relu(Mtilde)
            wst = np.stack([ws, wt], 1) * LEAKY                     # [64,2]
            c[f'wcol_{br}{i}'] = np.ascontiguousarray(np.concatenate([wst, wst], 0), np.float32)
            # replicated: cols 0-63 = ws_scaled, 64-127 = wt_scaled
            rep = np.concatenate([np.tile(wst[:, 0:1], (1, 64)),
                                  np.tile(wst[:, 1:2], (1, 64))], 1)
            c[f'wrep_{br}{i}'] = np.ascontiguousarray(np.concatenate([rep, rep], 0), np.float32)
    c['wp_s0'] = np.asarray(inputs['Wsp0'], np.float32)             # [64,12]
    c['wp_s1'] = np.asarray(inputs['Wsp1'], np.float32)             # [64,5]
    c['wp_t0'] = np.asarray(inputs['Wtp0'], np.float32)             # [64,32]
    c['wp_t1'] = np.asarray(inputs['Wtp1'], np.float32)             # [64,16]
    # bias rows replicated across partitions [64, n1]
    c['b_s0'] = np.tile(np.asarray(inputs['bsp0'], np.float32)[None, :], (64, 1))
    c['b_s1'] = np.tile(np.asarray(inputs['bsp1'], np.float32)[None, :], (64, 1))
    c['b_t0'] = np.tile(np.asarray(inputs['btp0'], np.float32)[None, :], (64, 1))
    c['b_t1'] = np.tile(np.asarray(inputs['btp1'], np.float32)[None, :], (64, 1))
    c['ident'] = np.concatenate([np.eye(64, dtype=np.float32)] * 2, 0)
    return c


CONST_SHAPES = {
    **{f'wcol_{b}{i}': (128, 2) for b in 'st' for i in range(3)},
    **{f'wrep_{b}{i}': (128, 128) for b in 'st' for i in range(3)},
    'wp_s0': (64, 12), 'wp_s1': (64, 5), 'wp_t0': (64, 32), 'wp_t1': (64, 16),
    'b_s0': (64, 12), 'b_s1': (64, 5), 'b_t0': (64, 32), 'b_t1': (64, 16),
    'ident': (128, 64),
}


def _lrelu_attn(nc, pools, rows_psum, cols_sb, col_idx_stride, out_sb, P_rows, G, W):
    """attn^T tile [P_rows, G*W] = lrelu(s+t) given Mtilde parts.

    rows_psum: [P_rows, G*W] psum AP holding 0.2*s rows (free-varying)
    cols_sb:   SBUF AP [P_rows, G] holding 0.2*t cols per graph
    out_sb:    [P_rows, G*W] SBUF tile for attn result
    """
    import concourse.mybir as mybir
    sb = pools['work']
    m = sb.tile([P_rows, G * W], F32, tag=f'mt{P_rows}_{W}')
    nc.vector.tensor_tensor(
        out=m[:].rearrange("p (g w) -> p g w", g=G),
        in0=rows_psum.rearrange("p (g w) -> p g w", g=G),
        in1=cols_sb.unsqueeze(2).broadcast_to([P_rows, G, W]),
        op=mybir.AluOpType.add)
    r = sb.tile([P_rows, G * W], F32, tag=f'r4{P_rows}_{W}')
    nc.scalar.activation(r[:], m[:], mybir.ActivationFunctionType.Relu, scale=4.0)
    nc.gpsimd.tensor_tensor(out=out_sb[:], in0=m[:], in1=r[:], op=mybir.AluOpType.add)


def _softmax(nc, pools, z_psum, brep, n1, G, s_out, tagp):
    """Row softmax over z (+bias brep) : z_psum [P, G*n1] -> s_out SBUF."""
    import concourse.mybir as mybir
    sb = pools['work']
    P = z_psum.shape[0]
    if brep is not None:
        zb = sb.tile([P, G * n1], F32, tag=f'zb{tagp}')
        nc.vector.tensor_tensor(
            out=zb[:].rearrange("p (g w) -> p g w", g=G),
            in0=z_psum.rearrange("p (g w) -> p g w", g=G),
            in1=brep[0:P, :].unsqueeze(1).broadcast_to([P, G, n1]),
            op=mybir.AluOpType.add)
        zsrc = zb[:]
    else:
        zsrc = z_psum
    nm = sb.tile([P, G], F32, tag=f'nm{tagp}')
    nc.vector.tensor_reduce(nm[:], zsrc.rearrange("p (g w) -> p g w", g=G),
                            axis=mybir.AxisListType.X, op=mybir.AluOpType.max,
                            negate=True)
    zs = sb.tile([P, G * n1], F32, tag=f'zs{tagp}')
    nc.vector.tensor_tensor(
        out=zs[:].rearrange("p (g w) -> p g w", g=G),
        in0=zsrc.rearrange("p (g w) -> p g w", g=G),
        in1=nm[:].unsqueeze(2).broadcast_to([P, G, n1]),
        op=mybir.AluOpType.add)
    nc.scalar.activation(zs[:], zs[:], mybir.ActivationFunctionType.Exp)
    r = sb.tile([P, G], F32, tag=f'sr{tagp}')
    nc.vector.tensor_reduce(r[:], zs[:].rearrange("p (g w) -> p g w", g=G),
                            axis=mybir.AxisListType.X, op=mybir.AluOpType.add)
    nc.vector.reciprocal(r[:], r[:])
    nc.vector.tensor_tensor(
        out=s_out[:].rearrange("p (g w) -> p g w", g=G),
        in0=zs[:].rearrange("p (g w) -> p g w", g=G),
        in1=r[:].unsqueeze(2).broadcast_to([P, G, n1]),
        op=mybir.AluOpType.mult)


def _branch_block(nc, pools, cst, br, xnm_views, xcm_views, out_stage, out_off,
                  G, scales, ident, cbase=0):
    """Process one block of G graphs for branch br ('s'|'t').

    xnm_views[g]: SBUF AP [n, 64] node-major x
    xcm_views[g]: SBUF AP [64, n] channel-major x (for stCols/sRep)
    out_stage: SBUF staging AP [n, ...]; out_off: column offset (elements)
    """
    import concourse.mybir as mybir
    n0, n1, n2 = scales
    sb, ps = pools['work'], pools['psum']

    # ---- scale 0 reps + cols ----
    strep = ps.tile([128, G * n0], F32, tag='rep')
    for g in range(G):
        nc.tensor.matmul(strep[:, g * n0:(g + 1) * n0],
                         cst[f'wrep_{br}0'][cbase:cbase + 64, :],
                         xcm_views[g], start=True, stop=True)
    stc = ps.tile([n0, 2 * G], F32, tag='pstc')
    for g in range(G):
        nc.tensor.matmul(stc[:, 2 * g:2 * g + 2], xcm_views[g],
                         cst[f'wcol_{br}0'][cbase:cbase + 64, :], start=True, stop=True)
    stcs = sb.tile([n0, 2 * G], F32, tag='stcs')
    nc.scalar.copy(stcs[:], stc[:])
    scols = stcs[:].rearrange("p (g two) -> p g two", two=2)[:, :, 0]
    tcols = stcs[:].rearrange("p (g two) -> p g two", two=2)[:, :, 1]

    attn0T = sb.tile([n0, G * n0], F32, tag='attn0T')
    _lrelu_attn(nc, pools, strep[0:n0, :], tcols, 2, attn0T, n0, G, n0)

    # ---- A0x^T, Z0, S0 ----
    a0 = ps.tile([64, G * n0], F32, tag='pA')
    for g in range(G):
        nc.tensor.matmul(a0[:, g * n0:(g + 1) * n0], xnm_views[g],
                         attn0T[:, g * n0:(g + 1) * n0], start=True, stop=True)
    a0s = sb.tile([64, G * n0], F32, tag='a0s')
    nc.scalar.copy(a0s[:], a0[:])
    z0 = ps.tile([n0, G * n1], F32, tag='pZ')
    for g in range(G):
        nc.tensor.matmul(z0[:, g * n1:(g + 1) * n1], a0s[:, g * n0:(g + 1) * n0],
                         cst[f'wp_{br}0'][:], start=True, stop=True)
    s0 = sb.tile([n0, G * n1], F32, tag='s0')
    _softmax(nc, pools, z0[:], cst[f'b_{br}0'], n1, G, s0, 'z0')

    # S0^T
    s0t_p = ps.tile([n1, G * n0], F32, tag='pB')
    for g in range(G):
        nc.tensor.transpose(s0t_p[:, g * n0:(g + 1) * n0],
                            s0[:, g * n1:(g + 1) * n1], ident[0:n0, 0:n0])
    s0t = sb.tile([n1, G * n0], F32, tag='s0ts')
    nc.scalar.copy(s0t[:], s0t_p[:])

    # x1 = S0^T @ X  (node-major [n1, 64])
    x1_p = ps.tile([n1, G * 64], F32, tag='pC')
    for g in range(G):
        nc.tensor.matmul(x1_p[:, g * 64:(g + 1) * 64], s0[:, g * n1:(g + 1) * n1],
                         xnm_views[g], start=True, stop=True)
    x1 = sb.tile([n1, G * 64], F32, tag='x1s')
    nc.scalar.copy(x1[:], x1_p[:])
    # x1 channel-major
    x1c_p = ps.tile([64, G * n1], F32, tag='pB')
    for g in range(G):
        nc.tensor.transpose(x1c_p[:, g * n1:(g + 1) * n1],
                            x1[:, g * 64:(g + 1) * 64], ident[0:n1, 0:n1])
    x1c = sb.tile([64, G * n1], F32, tag='x1cs')
    nc.scalar.copy(x1c[:], x1c_p[:])

    # ---- scale 1 ----
    strep1 = ps.tile([128, G * n1], F32, tag='rep')
    for g in range(G):
        nc.tensor.matmul(strep1[:, g * n1:(g + 1) * n1],
                         cst[f'wrep_{br}1'][0:64, :],
                         x1c[:, g * n1:(g + 1) * n1], start=True, stop=True)
    stc1 = ps.tile([n1, 2 * G], F32, tag='pstc')
    for g in range(G):
        nc.tensor.matmul(stc1[:, 2 * g:2 * g + 2], x1c[:, g * n1:(g + 1) * n1],
                         cst[f'wcol_{br}1'][0:64, :], start=True, stop=True)
    stc1s = sb.tile([n1, 2 * G], F32, tag='stc1s')
    nc.scalar.copy(stc1s[:], stc1[:])
    tcols1 = stc1s[:].rearrange("p (g two) -> p g two", two=2)[:, :, 1]
    attn1T = sb.tile([n1, G * n1], F32, tag='attn1T')
    _lrelu_attn(nc, pools, strep1[0:n1, :], tcols1, 2, attn1T, n1, G, n1)

    a1 = ps.tile([64, G * n1], F32, tag='pA')
    for g in range(G):
        nc.tensor.matmul(a1[:, g * n1:(g + 1) * n1], x1[:, g * 64:(g + 1) * 64],
                         attn1T[:, g * n1:(g + 1) * n1], start=True, stop=True)
    a1s = sb.tile([64, G * n1], F32, tag='a1s')
    nc.scalar.copy(a1s[:], a1[:])
    z1 = ps.tile([n1, G * n2], F32, tag='pZ')
    for g in range(G):
        nc.tensor.matmul(z1[:, g * n2:(g + 1) * n2], a1s[:, g * n1:(g + 1) * n1],
                         cst[f'wp_{br}1'][:], start=True, stop=True)
    s1 = sb.tile([n1, G * n2], F32, tag='s1')
    _softmax(nc, pools, z1[:], cst[f'b_{br}1'], n2, G, s1, 'z1')

    s1t_p = ps.tile([n2, G * n1], F32, tag='s1t')
    for g in range(G):
        nc.tensor.transpose(s1t_p[:, g * n1:(g + 1) * n1],
                            s1[:, g * n2:(g + 1) * n2], ident[0:n1, 0:n1])
    s1t = sb.tile([n2, G * n1], F32, tag='s1ts')
    nc.scalar.copy(s1t[:], s1t_p[:])

    # x2 = S1^T @ x1 (node-major [n2, 64])
    x2_p = ps.tile([n2, G * 64], F32, tag='pC')
    for g in range(G):
        nc.tensor.matmul(x2_p[:, g * 64:(g + 1) * 64], s1[:, g * n2:(g + 1) * n2],
                         x1[:, g * 64:(g + 1) * 64], start=True, stop=True)
    x2 = sb.tile([n2, G * 64], F32, tag='x2s')
    nc.scalar.copy(x2[:], x2_p[:])
    x2c_p = ps.tile([64, G * n2], F32, tag='pB')
    for g in range(G):
        nc.tensor.transpose(x2c_p[:, g * n2:(g + 1) * n2],
                            x2[:, g * 64:(g + 1) * 64], ident[0:n2, 0:n2])
    x2c = sb.tile([64, G * n2], F32, tag='x2cs')
    nc.scalar.copy(x2c[:], x2c_p[:])

    # ---- scale 2 attn ----
    strep2 = ps.tile([128, G * n2], F32, tag='rep')
    for g in range(G):
        nc.tensor.matmul(strep2[:, g * n2:(g + 1) * n2],
                         cst[f'wrep_{br}2'][0:64, :],
                         x2c[:, g * n2:(g + 1) * n2], start=True, stop=True)
    stc2 = ps.tile([n2, 2 * G], F32, tag='pstc')
    for g in range(G):
        nc.tensor.matmul(stc2[:, 2 * g:2 * g + 2], x2c[:, g * n2:(g + 1) * n2],
                         cst[f'wcol_{br}2'][0:64, :], start=True, stop=True)
    stc2s = sb.tile([n2, 2 * G], F32, tag='stc2s')
    nc.scalar.copy(stc2s[:], stc2[:])
    tcols2 = stc2s[:].rearrange("p (g two) -> p g two", two=2)[:, :, 1]
    attn2T = sb.tile([n2, G * n2], F32, tag='attn2T')
    _lrelu_attn(nc, pools, strep2[0:n2, :], tcols2, 2, attn2T, n2, G, n2)

    # ---- fusion: inner^T = attn1T + (S1 attn2T) S1T ; F = attn0 + S0 inner S0T
    dt_p = ps.tile([n2, G * n1], F32, tag='pA')
    for g in range(G):
        nc.tensor.matmul(dt_p[:, g * n1:(g + 1) * n1],
                         attn2T[:, g * n2:(g + 1) * n2],
                         s1t[:, g * n1:(g + 1) * n1], start=True, stop=True)
    dts = sb.tile([n2, G * n1], F32, tag='dts')
    nc.scalar.copy(dts[:], dt_p[:])
    it_p = ps.tile([n1, G * n1], F32, tag='pB')
    for g in range(G):
        nc.tensor.matmul(it_p[:, g * n1:(g + 1) * n1],
                         dts[:, g * n1:(g + 1) * n1],
                         s1t[:, g * n1:(g + 1) * n1], start=True, stop=True)
    its = sb.tile([n1, G * n1], F32, tag='its')
    nc.vector.tensor_tensor(out=its[:], in0=attn1T[:], in1=it_p[:],
                            op=mybir.AluOpType.add)
    ap_p = ps.tile([n1, G * n0], F32, tag='pC')
    for g in range(G):
        nc.tensor.matmul(ap_p[:, g * n0:(g + 1) * n0],
                         its[:, g * n1:(g + 1) * n1],
                         s0t[:, g * n0:(g + 1) * n0], start=True, stop=True)
    aps = sb.tile([n1, G * n0], F32, tag='apfs')
    nc.scalar.copy(aps[:], ap_p[:])
    f_p = ps.tile([n0, G * n0], F32, tag='pF')
    for g in range(G):
        nc.tensor.matmul(f_p[:, g * n0:(g + 1) * n0],
                         s0t[:, g * n0:(g + 1) * n0],
                         aps[:, g * n0:(g + 1) * n0], start=True, stop=False)
        nc.tensor.matmul(f_p[:, g * n0:(g + 1) * n0],
                         attn0T[:, g * n0:(g + 1) * n0],
                         ident[0:n0, 0:n0], is_transpose=True,
                         start=False, stop=True, skip_group_check=False)
    _softmax(nc, pools, f_p[:], None, n0, G,
             out_stage[:, out_off:out_off + G * n0], 'f' + br)


def build_program(nc, tc, ctx, n_pairs, use_loop):
    """Emit the full per-core program into nc/tc."""
    import concourse.mybir as mybir
    import concourse.bass as bass
    import concourse.tile as tile
    global F32
    F32 = mybir.dt.float32

    src = nc.dram_tensor("src", (BPC, C, T, J), F32, kind="ExternalInput").ap()
    sa = nc.dram_tensor("sa", (BPC, T, J, J), F32, kind="ExternalOutput").ap()
    ta = nc.dram_tensor("ta", (BPC, J, T, T), F32, kind="ExternalOutput").ap()
    cst_dram = {k: nc.dram_tensor(k, v, F32, kind="ExternalInput").ap()
                for k, v in CONST_SHAPES.items()}

    const_pool = ctx.enter_context(tc.tile_pool(name="const", bufs=1))
    io_pool = ctx.enter_context(tc.tile_pool(name="io", bufs=2))
    work = ctx.enter_context(tc.tile_pool(name="work", bufs=2))
    psum = ctx.enter_context(tc.tile_pool(name="psum", bufs=1, space="PSUM"))
    pools = {'work': work, 'psum': psum}

    cst = {}
    for k, shp in CONST_SHAPES.items():
        t = const_pool.tile(list(shp), F32, tag=k)
        nc.sync.dma_start(t[:], cst_dram[k][:])
        cst[k] = t
    ident = cst['ident']

    def body(pair_i):
        # ---- loads ----
        slab = io_pool.tile([128, T * J], F32, tag='slab')
        nc.sync.dma_start(slab[:], src[bass.ds(pair_i, 2)]
                          .rearrange("b c t j -> (b c) (t j)"))
        tpnm = io_pool.tile([T, 2 * C * J], F32, tag='tpnm')
        nc.sync.dma_start(tpnm[:].rearrange("t (b c j) -> t b c j", b=2, c=C),
                          src[bass.ds(pair_i, 2)].transpose([2, 0, 1, 3]))
        sa_stage = io_pool.tile([J, 2 * T * J], F32, tag='sastage')
        ta_stage = io_pool.tile([T, 2 * J * T], F32, tag='tastage')

        slab_v = slab[:].rearrange("p (t j) -> p t j", j=J)
        tpnm_v = tpnm[:].rearrange("p (b c j) -> p b c j", b=2, j=J)

        # ---- temporal branch: graphs (bl, j), blocks of G=5 over j ----
        Gt = 5
        for bl in range(2):
            for j0 in range(0, J, Gt):
                xnm = [tpnm_v[:, bl, :, j0 + g] for g in range(Gt)]
                # xnm view: [T, C] with free stride J
                xcm = [slab_v[bl * 64:(bl + 1) * 64, :, j0 + g] for g in range(Gt)]
                off = bl * (J * T) + j0 * T
                _branch_block(nc, pools, cst, 't', xnm, xcm, ta_stage, off,
                              Gt, T_SCALES, ident, cbase=bl * 64)
        # ---- spatial branch: graphs (bl, t), blocks of G=8 over t ----
        Gs = 8
        for bl in range(2):
            for t0 in range(0, T, Gs):
                xcm = [slab_v[bl * 64:(bl + 1) * 64, t0 + g, :] for g in range(Gs)]
                # x node-major via PE transpose of xcm
                xr_p = psum.tile([J, Gs * 64], F32, tag='rep')
                for g in range(Gs):
                    nc.tensor.transpose(xr_p[:, g * 64:(g + 1) * 64], xcm[g],
                                        ident[bl * 64:(bl + 1) * 64, :])
                xr = work.tile([J, Gs * 64], F32, tag='spxnms')
                nc.scalar.copy(xr[:], xr_p[:])
                xnm = [xr[:, g * 64:(g + 1) * 64] for g in range(Gs)]
                off = bl * (T * J) + t0 * J
                _branch_block(nc, pools, cst, 's', xnm, xcm, sa_stage, off,
                              Gs, S_SCALES, ident, cbase=bl * 64)

        # ---- stores ----
        nc.sync.dma_start(sa[bass.ds(pair_i, 2)].transpose([2, 0, 1, 3]),
                          sa_stage[:].rearrange("i (b t j) -> i b t j", b=2, t=T))
        nc.sync.dma_start(ta[bass.ds(pair_i, 2)].transpose([2, 0, 1, 3]),
                          ta_stage[:].rearrange("i (b j k) -> i b j k", b=2, j=J))

    if use_loop:
        with tc.For_i(0, 2 * n_pairs, 2) as i:
            body(i)
    else:
        for p in range(n_pairs):
            body(2 * p)


import concourse.mybir as mybir  # noqa: E402


def _build_nc(n_pairs=NPAIR, use_loop=True):
    from contextlib import ExitStack
    import concourse.bass as bass
    import concourse.tile as tile
    from concourse import bacc
    from concourse._compat import axon_active
    nc = bacc.Bacc("TRN2", target_bir_lowering=False, debug=False,
                   enable_asserts=False, num_devices=1)
    with tile.TileContext(nc) as tc:
        with ExitStack() as ctx:
            build_program(nc, tc, ctx, n_pairs, use_loop)
    nc.compile()
    return nc


_NC_CACHE = {}


def kernel(**inputs):
    from concourse.bass_utils import run_bass_kernel_spmd
    key = 'full'
    if key not in _NC_CACHE:
        _NC_CACHE[key] = _build_nc()
    nc = _NC_CACHE[key]
    cst = _host_consts(inputs)
    src = np.ascontiguousarray(np.asarray(inputs['src'], np.float32))
    in_maps = []
    for c in range(NCORES):
        m = {'src': np.ascontiguousarray(src[c * BPC:(c + 1) * BPC])}
        m.update(cst)
        in_maps.append(m)
    res = run_bass_kernel_spmd(nc, in_maps, core_ids=list(range(NCORES)))
    sa = np.concatenate([r['sa'] for r in res.results], 0)
    ta = np.concatenate([r['ta'] for r in res.results], 0)
    return sa, ta
